# revision 20
# baseline (speedup 1.0000x reference)
"""Trainium2 Bass kernel for nn_ConvAttention: LayerNorm -> 1x1-conv QKV ->
per-(b,h)-row attention over W -> skip connection.

Sharding: data-parallel over batch B=8 across 8 NeuronCores. Each core
processes 64 (h) slabs of [W=256, C=256].

Numerics strategy: all matmuls run in float32r (TF32-like, ~13 effective
mantissa bits, measured) at 1 cycle/row for >=256-col outputs -- within
~27% of bf16 rate but with no hi/lo splits needed. Score error ~2^-9
absolute, far inside the 2e-2 relative gate.

Softmax max-subtraction is replaced by a constant shift (exact in real
arithmetic; scores are bounded well inside fp32 exp range), computing only
transposed scores s^T = k @ q^T and exponentiating directly. Z comes from a
ones column appended to the V operand of the output matmul.
"""

import os
import sys

for _p in ("/opt/trn_rl_repo", "/root/.axon_site/_ro/trn_rl_repo"):
    if _p not in sys.path:
        sys.path.insert(0, _p)

import numpy as np

import concourse.tile as tile
from concourse import bacc, mybir
from concourse.bass_utils import run_bass_kernel_spmd

F32 = mybir.dt.float32
F32R = mybir.dt.float32r
BF16 = mybir.dt.bfloat16
AF = mybir.ActivationFunctionType
ALU = mybir.AluOpType

B, H, W, C = 8, 64, 256, 256
F2 = 2 * C
NS = H  # slabs per core (batch-sharded over 8 cores)
EPS = 1e-3  # Keras LayerNormalization default
SHIFT = 32.0  # constant softmax shift (replaces per-row max subtraction)

_NC_CACHE: dict = {}


def _install_act_root():
    """Reorder act_info.json so natural_log_exp_and_others is the first set:
    bass' first-match table chooser then resolves both Ln and Exp to that one
    set instead of alternating exp_and_others / natural_log every slab."""
    if os.environ.get("BASS_ACT_ROOT_JSON_PATH"):
        return
    try:
        import json
        import tempfile

        import neuronxcc.driver.jobs.support.FindActInfo as FAI
        from neuronxcc.driver.Job import Job

        src = FAI.findActInfoFile(Job.getPackageDir(), "gen3")
        srcdir = os.path.dirname(src)
        d = json.load(open(src))
        sets = d["act_func_sets"]
        first = [s for s in sets if s["name"] == "natural_log_exp_and_others"]
        if not first:
            return
        rest = [s for s in sets if s["name"] != "natural_log_exp_and_others"]
        d["act_func_sets"] = first + rest
        td = tempfile.mkdtemp(prefix="act_root_")
        for fn in os.listdir(srcdir):
            sp = os.path.join(srcdir, fn)
            if os.path.isfile(sp) and fn != os.path.basename(src):
                os.symlink(sp, os.path.join(td, fn))
        out = os.path.join(td, os.path.basename(src))
        with open(out, "w") as f:
            json.dump(d, f)
        os.environ["BASS_ACT_ROOT_JSON_PATH"] = out
        _orig = FAI.findActInfoFile
        FAI.findActInfoFile = lambda *a, **k: out
        import concourse.hw_specs as hw_specs

        hw_specs.get_activation_tables.cache_clear()
    except Exception as e:  # noqa: BLE001
        print(f"act root override failed (table thrash will persist): {e}")


def _build(with_bias: bool):
    _install_act_root()
    nc = bacc.Bacc("TRN2", target_bir_lowering=False, debug=False, num_devices=8)
    x_d = nc.dram_tensor("x", [NS, W, C], F32R, kind="ExternalInput").ap()
    wqk_d = nc.dram_tensor("wqk", [2, 128, 256], F32R, kind="ExternalInput").ap()
    wv_d = nc.dram_tensor("wv", [2, 128, 256], F32R, kind="ExternalInput").ap()
    ident_d = nc.dram_tensor("ident_in", [128, 128], F32R, kind="ExternalInput").ap()
    onesc_d = nc.dram_tensor("onesc_in", [128, 4], BF16, kind="ExternalInput").ap()
    bqk_d = bv_d = None
    if with_bias:
        bqk_d = nc.dram_tensor("bqk", [2, 128], F32, kind="ExternalInput").ap()
        bv_d = nc.dram_tensor("bv", [256], F32, kind="ExternalInput").ap()
    out_d = nc.dram_tensor("out", [NS, W, C], F32, kind="ExternalOutput").ap()

    # per-slab views: [p=128, t(w-chunk)=2, c=256]
    x_r = x_d.rearrange("s (t p) c -> s p t c", p=128)
    out_r = out_d.rearrange("s (t p) c -> s p t c", p=128)

    with tile.TileContext(nc) as tc:
        _emit(nc, tc, x_r, out_r, wqk_d, wv_d, ident_d, onesc_d, bqk_d, bv_d)
    nc.compile()
    return nc


def _emit(nc, tc, x_r, out_r, wqk_d, wv_d, ident_d, onesc_d, bqk_d, bv_d):
    from contextlib import ExitStack

    with ExitStack() as ctx:
        ec = ctx.enter_context
        consts = ec(tc.tile_pool(name="consts", bufs=1))
        xpool = ec(tc.tile_pool(name="xp", bufs=6))
        xnpool = ec(tc.tile_pool(name="xnp", bufs=3))
        xtpool = ec(tc.tile_pool(name="xtp", bufs=3))
        qkpool = ec(tc.tile_pool(name="qkp", bufs=3))
        epool = ec(tc.tile_pool(name="ep", bufs=3))
        vpool = ec(tc.tile_pool(name="vp", bufs=3))
        opool = ec(tc.tile_pool(name="op", bufs=4))
        stat = ec(tc.tile_pool(name="stat", bufs=7))
        ps_xnT = ec(tc.tile_pool(name="ps_xnT", bufs=1, space="PSUM"))
        ps_qk = ec(tc.tile_pool(name="ps_qk", bufs=2, space="PSUM"))
        ps_sT = ec(tc.tile_pool(name="ps_sT", bufs=1, space="PSUM"))
        ps_v = ec(tc.tile_pool(name="ps_v", bufs=2, space="PSUM"))
        ps_y = ec(tc.tile_pool(name="ps_y", bufs=1, space="PSUM"))

        ident = consts.tile([128, 128], F32R)
        nc.sync.dma_start(ident, ident_d)
        onesc = consts.tile([128, 2, 2], BF16)
        nc.sync.dma_start(onesc, onesc_d.rearrange("p (t k) -> p t k", t=2))
        negshift = consts.tile([128, 1], F32)
        nc.vector.memset(negshift, -SHIFT)
        eps_t = consts.tile([128, 1], F32)
        nc.vector.memset(eps_t, EPS)

        wqk = consts.tile([128, 2, 256], F32R)
        nc.sync.dma_start(wqk, wqk_d.rearrange("t p f -> p t f"))
        wv = consts.tile([128, 2, 256], F32R)
        nc.sync.dma_start(wv, wv_d.rearrange("t p f -> p t f"))

        if bqk_d is not None:
            bqk_sb = consts.tile([128, 2], F32)
            nc.sync.dma_start(bqk_sb, bqk_d.rearrange("t p -> p t"))
            import concourse.bass as bass
            bvf = consts.tile([128, 2, 256], F32)
            bv_b = bass.AP(tensor=bv_d.tensor, offset=bv_d.offset,
                           ap=[[0, 128], [0, 2], [1, 256]])
            nc.sync.dma_start(bvf, bv_b)

        def emit_tail(pv):
            """Slab tail, software-pipelined: y-matmuls, 1/Z normalize,
            skip-add, store."""
            E, vt, x_sb, s = pv["E"], pv["vt"], pv["x_sb"], pv["s"]
            p_y = ps_y.tile([128, 2, 512], F32)
            for it in (0, 1):
                for jt in (0, 1):
                    nc.tensor.matmul(
                        p_y[:, it, 0:258],
                        E[:, jt, it * 128:(it + 1) * 128],
                        vt[:, jt, 0:258],
                        start=(jt == 0), stop=(jt == 1))
            rZ = stat.tile([128, 2, 1], F32)
            nc.vector.reciprocal(rZ, p_y[:, :, 256:257])
            # out = x + y * rZ
            tmp = opool.tile([128, 2, 256], F32)
            for it in (0, 1):
                nc.scalar.mul(tmp[:, it, :], p_y[:, it, 0:256], rZ[:, it, :])
            o_sb = opool.tile([128, 2, 256], F32)
            nc.gpsimd.tensor_tensor(out=o_sb, in0=tmp, in1=x_sb, op=ALU.add)
            nc.sync.dma_start(out_r[s], o_sb)

        def emit_scores(pq):
            """Scores stage, software-pipelined: s^T matmuls, exp."""
            qT = pq["qT"]
            p_sT = ps_sT.tile([128, 2, 256], F32)
            for jt in (0, 1):
                nc.tensor.matmul(
                    p_sT[:, jt, :],
                    qT[:, 1, jt * 128:(jt + 1) * 128],
                    qT[:, 0, :],
                    start=True, stop=True)
            E = epool.tile([128, 2, 256], BF16)
            nc.scalar.activation(out=E, in_=p_sT, func=AF.Exp,
                                 bias=negshift, scale=1.0)
            return {"E": E, "vt": pq["vt"], "x_sb": pq["x_sb"], "s": pq["s"]}

        def emit_front(s):
            """LN front-end for slab s: DMA, stats, rsqrt, normalize.
            Emitted one iteration ahead so the V/S LN chain never gates
            the slab's first PE op (the transposes)."""
            x_sb = xpool.tile([128, 2, 256], F32R)
            nc.sync.dma_start(x_sb, x_r[s])

            # LayerNorm stats per row (partition = w position)
            st = stat.tile([128, 2, 6], F32)
            mv = stat.tile([128, 2, 2], F32)
            for t in (0, 1):
                nc.vector.bn_stats(st[:, t, :], x_sb[:, t, :])
                nc.vector.bn_aggr(mv[:, t, :], st[:, t, :])
            # rs = rsqrt(var + eps) = exp(-0.5 * ln(var + eps))
            lnv = stat.tile([128, 2, 1], F32)
            nc.scalar.activation(out=lnv, in_=mv[:, :, 1:2], func=AF.Ln,
                                 bias=eps_t, scale=1.0)
            rs = stat.tile([128, 2, 1], F32)
            nc.scalar.activation(out=rs, in_=lnv, func=AF.Exp, scale=-0.5)

            # xn = (x - mean) * rs   (gamma/beta folded into weights on host)
            xn = xnpool.tile([128, 2, 256], F32R)
            for t in (0, 1):
                nc.vector.tensor_scalar(
                    out=xn[:, t, :], in0=x_sb[:, t, :],
                    scalar1=mv[:, t, 0:1], scalar2=rs[:, t, :],
                    op0=ALU.subtract, op1=ALU.mult)
            return {"x_sb": x_sb, "xn": xn}

        def emit_transp(fr):
            """PE transpose + PSUM->SBUF drain for a slab whose LN front-end
            was emitted earlier. Emitted one iteration ahead of consumption so
            qk/v never wait on same-iteration scalar-engine work."""
            xn = fr["xn"]
            p_xnT = ps_xnT.tile([128, 2, 256], F32R)
            for cc in (0, 1):
                for t in (0, 1):
                    nc.tensor.transpose(
                        p_xnT[:, cc, t * 128:(t + 1) * 128],
                        xn[:, t, cc * 128:(cc + 1) * 128], ident)
            xT = xtpool.tile([128, 2, 256], F32R)
            nc.vector.tensor_copy(xT, p_xnT)
            return {"x_sb": fr["x_sb"], "xT": xT}

        # persistent vt ring: ones columns are pre-filled once; per-slab
        # copies only touch cols 0:256, so the Z ones never need rewriting
        vts = []
        for r in range(3):
            vt_r = vpool.tile([128, 2, 258], BF16)
            nc.vector.tensor_copy(vt_r[:, :, 256:258], onesc)
            vts.append(vt_r)

        # Steady-state PE order per iteration s:
        #   Y(s-2), SC(s-1), QK(s), V(s), T(s+1)
        # Every PE group consumes only elementwise products emitted in a
        # PREVIOUS iteration (except T <- xn, which has ~2us of V slack), so
        # the PE never waits on same-iteration Vector/Scalar work and stays
        # at max p-state.
        prev = None
        prevq = None
        cur = emit_transp(emit_front(0))
        front_next = emit_front(1)
        for s in range(NS):
            x_sb, xT = cur["x_sb"], cur["xT"]

            # front for s+2 FIRST: V leads with stats/xn and S with lnv/rs,
            # so neither sits behind PE-gated copies in its queue
            new_front = emit_front(s + 2) if s + 2 < NS else None

            if prev is not None:
                emit_tail(prev)
            if prevq is not None:
                prev = emit_scores(prevq)

            # T(s+1) in the MIDDLE of the PE order: its xT drain then has
            # QK(s)+V(s) plus the next iteration's Y+SC of PE cover before
            # QK(s+1) consumes it
            nxt_cur = emit_transp(front_next) if s + 1 < NS else None
            front_next = new_front

            # qk^T = Wqk^T @ xn^T
            p_qk = ps_qk.tile([128, 2, 256], F32)
            for blk in (0, 1):
                for cc in (0, 1):
                    nc.tensor.matmul(
                        p_qk[:, blk, :],
                        wqk[:, cc, blk * 128:(blk + 1) * 128],
                        xT[:, cc, :],
                        start=(cc == 0), stop=(cc == 1))
            if bqk_d is not None:
                for blk in (0, 1):
                    nc.vector.tensor_scalar(
                        out=p_qk[:, blk, :], in0=p_qk[:, blk, :],
                        scalar1=bqk_sb[:, blk:blk + 1], scalar2=None,
                        op0=ALU.add)
            qT = qkpool.tile([128, 2, 256], F32R)
            nc.scalar.copy(qT, p_qk)

            # v = xn @ Wv, with a ones column appended for Z accumulation
            p_v = ps_v.tile([128, 2, 256], F32)
            for jt in (0, 1):
                for cc in (0, 1):
                    nc.tensor.matmul(
                        p_v[:, jt, :],
                        xT[:, cc, jt * 128:(jt + 1) * 128],
                        wv[:, cc, :],
                        start=(cc == 0), stop=(cc == 1))
            vt = vts[s % 3]
            if bv_d is not None:
                nc.vector.tensor_tensor(out=vt[:, :, 0:256], in0=p_v, in1=bvf,
                                        op=ALU.add)
            else:
                nc.scalar.copy(vt[:, :, 0:256], p_v)

            cur = nxt_cur

            prevq = {"qT": qT, "vt": vt, "x_sb": x_sb, "s": s}
        emit_tail(prev)
        prev = emit_scores(prevq)
        emit_tail(prev)


def _install_ntff_hook():
    """Register the axon NTFF profiling hook. Trace-only; best-effort."""
    try:
        import types

        import antenv

        if getattr(antenv, "axon_hooks", None) is not None:
            return
        mod = types.ModuleType("antenv.axon_hooks")
        _h = [None]
        mod.set_axon_ntff_profile_hook = lambda h: _h.__setitem__(0, h)
        mod.get_axon_ntff_profile_hook = lambda: _h[0]
        sys.modules["antenv.axon_hooks"] = mod
        antenv.axon_hooks = mod
        from trn_agent_boot.trn_boot import _ntff_profile_via_ctypes

        hook = _ntff_profile_via_ctypes("/opt/axon/libaxon_pjrt.so")
        if hook is not None:
            mod.set_axon_ntff_profile_hook(hook)
    except Exception as e:  # noqa: BLE001
        print(f"ntff hook install failed (timing unavailable): {e}")


def kernel(x, ln_gamma, ln_beta, W_qkv):
    x = np.asarray(x, dtype=np.float32)
    ln_gamma = np.asarray(ln_gamma, dtype=np.float32)
    ln_beta = np.asarray(ln_beta, dtype=np.float32)
    W_qkv = np.asarray(W_qkv, dtype=np.float32)
    assert x.shape == (B, H, W, C) and W_qkv.shape == (C, F2)

    # fold gamma/beta into the projection (1x1 conv has no bias of its own)
    Wp = (ln_gamma.astype(np.float64)[:, None] * W_qkv.astype(np.float64))
    bW = (ln_beta.astype(np.float64) @ W_qkv.astype(np.float64)).astype(np.float32)
    with_bias = bool(np.any(bW != 0.0))

    key = with_bias
    if key not in _NC_CACHE:
        _NC_CACHE[key] = _build(with_bias)
    nc = _NC_CACHE[key]

    wqk = np.ascontiguousarray(
        Wp[:, :256].astype(np.float32).reshape(2, 128, 256))
    wv = np.ascontiguousarray(
        Wp[:, 256:].astype(np.float32).reshape(2, 128, 256))
    ident = np.eye(128, dtype=np.float32)
    import ml_dtypes
    onesc = np.ones((128, 4), dtype=ml_dtypes.bfloat16)
    in_maps = []
    for b in range(B):
        m = {
            "x": np.ascontiguousarray(x[b]),
            "wqk": wqk,
            "wv": wv,
            "ident_in": ident,
            "onesc_in": onesc,
        }
        if with_bias:
            m["bqk"] = np.ascontiguousarray(bW[:256].reshape(2, 128))
            m["bv"] = np.ascontiguousarray(bW[256:])
        in_maps.append(m)

    trace = os.environ.get("KERNEL_TRACE", "") == "1"
    if trace:
        _install_ntff_hook()
    res = run_bass_kernel_spmd(nc, in_maps, core_ids=list(range(B)), trace=trace)
    if trace and res.exec_time_ns is not None:
        print(f"HW exec time: {res.exec_time_ns} ns")
        if res.instructions_and_trace is not None:
            print(f"trace: {res.instructions_and_trace[1]}")
    out = np.stack([res.results[b]["out"] for b in range(B)], axis=0)
    return out.reshape(B, H, W, C).astype(np.float32, copy=False)


# revision 21
# speedup vs baseline: 1.1134x; 1.1134x over previous
"""Trainium2 Bass kernel for nn_ConvAttention: LayerNorm -> 1x1-conv QKV ->
per-(b,h)-row attention over W -> skip connection.

Sharding: data-parallel over batch B=8 across 8 NeuronCores. Each core
processes 64 (h) slabs of [W=256, C=256].

Numerics strategy: all matmuls run in float32r (TF32-like, ~13 effective
mantissa bits, measured) at 1 cycle/row for >=256-col outputs -- within
~27% of bf16 rate but with no hi/lo splits needed. Score error ~2^-9
absolute, far inside the 2e-2 relative gate.

Softmax max-subtraction is replaced by a constant shift (exact in real
arithmetic; scores are bounded well inside fp32 exp range), computing only
transposed scores s^T = k @ q^T and exponentiating directly. Z comes from a
ones column appended to the V operand of the output matmul.
"""

import os
import sys

for _p in ("/opt/trn_rl_repo", "/root/.axon_site/_ro/trn_rl_repo"):
    if _p not in sys.path:
        sys.path.insert(0, _p)

import numpy as np

import concourse.tile as tile
from concourse import bacc, mybir
from concourse.bass_utils import run_bass_kernel_spmd

F32 = mybir.dt.float32
F32R = mybir.dt.float32r
BF16 = mybir.dt.bfloat16
AF = mybir.ActivationFunctionType
ALU = mybir.AluOpType

B, H, W, C = 8, 64, 256, 256
F2 = 2 * C
NS = H  # slabs per core (batch-sharded over 8 cores)
EPS = 1e-3  # Keras LayerNormalization default
SHIFT = 32.0  # constant softmax shift (replaces per-row max subtraction)

_NC_CACHE: dict = {}


def _install_act_root():
    """Reorder act_info.json so natural_log_exp_and_others is the first set:
    bass' first-match table chooser then resolves both Ln and Exp to that one
    set instead of alternating exp_and_others / natural_log every slab."""
    if os.environ.get("BASS_ACT_ROOT_JSON_PATH"):
        return
    try:
        import json
        import tempfile

        import neuronxcc.driver.jobs.support.FindActInfo as FAI
        from neuronxcc.driver.Job import Job

        src = FAI.findActInfoFile(Job.getPackageDir(), "gen3")
        srcdir = os.path.dirname(src)
        d = json.load(open(src))
        sets = d["act_func_sets"]
        first = [s for s in sets if s["name"] == "natural_log_exp_and_others"]
        if not first:
            return
        rest = [s for s in sets if s["name"] != "natural_log_exp_and_others"]
        d["act_func_sets"] = first + rest
        td = tempfile.mkdtemp(prefix="act_root_")
        for fn in os.listdir(srcdir):
            sp = os.path.join(srcdir, fn)
            if os.path.isfile(sp) and fn != os.path.basename(src):
                os.symlink(sp, os.path.join(td, fn))
        out = os.path.join(td, os.path.basename(src))
        with open(out, "w") as f:
            json.dump(d, f)
        os.environ["BASS_ACT_ROOT_JSON_PATH"] = out
        _orig = FAI.findActInfoFile
        FAI.findActInfoFile = lambda *a, **k: out
        import concourse.hw_specs as hw_specs

        hw_specs.get_activation_tables.cache_clear()
    except Exception as e:  # noqa: BLE001
        print(f"act root override failed (table thrash will persist): {e}")


def _build(with_bias: bool):
    _install_act_root()
    nc = bacc.Bacc("TRN2", target_bir_lowering=False, debug=False, num_devices=8)
    x_d = nc.dram_tensor("x", [NS, W, C], F32R, kind="ExternalInput").ap()
    wqk_d = nc.dram_tensor("wqk", [2, 128, 256], F32R, kind="ExternalInput").ap()
    wv_d = nc.dram_tensor("wv", [2, 128, 256], F32R, kind="ExternalInput").ap()
    ident_d = nc.dram_tensor("ident_in", [128, 128], F32R, kind="ExternalInput").ap()
    onesc_d = nc.dram_tensor("onesc_in", [128, 4], BF16, kind="ExternalInput").ap()
    bqk_d = bv_d = None
    if with_bias:
        bqk_d = nc.dram_tensor("bqk", [2, 128], F32, kind="ExternalInput").ap()
        bv_d = nc.dram_tensor("bv", [256], F32, kind="ExternalInput").ap()
    out_d = nc.dram_tensor("out", [NS, W, C], F32, kind="ExternalOutput").ap()

    # per-slab views: [p=128, t(w-chunk)=2, c=256]
    x_r = x_d.rearrange("s (t p) c -> s p t c", p=128)
    out_r = out_d.rearrange("s (t p) c -> s p t c", p=128)

    with tile.TileContext(nc) as tc:
        _emit(nc, tc, x_r, out_r, wqk_d, wv_d, ident_d, onesc_d, bqk_d, bv_d)
    nc.compile()
    return nc


def _emit(nc, tc, x_r, out_r, wqk_d, wv_d, ident_d, onesc_d, bqk_d, bv_d):
    from contextlib import ExitStack

    with ExitStack() as ctx:
        ec = ctx.enter_context
        consts = ec(tc.tile_pool(name="consts", bufs=1))
        xpool = ec(tc.tile_pool(name="xp", bufs=6))
        xnpool = ec(tc.tile_pool(name="xnp", bufs=3))
        xtpool = ec(tc.tile_pool(name="xtp", bufs=3))
        qkpool = ec(tc.tile_pool(name="qkp", bufs=3))
        epool = ec(tc.tile_pool(name="ep", bufs=3))
        vpool = ec(tc.tile_pool(name="vp", bufs=3))
        opool = ec(tc.tile_pool(name="op", bufs=4))
        stat = ec(tc.tile_pool(name="stat", bufs=7))
        ps_xnT = ec(tc.tile_pool(name="ps_xnT", bufs=1, space="PSUM"))
        ps_qk = ec(tc.tile_pool(name="ps_qk", bufs=2, space="PSUM"))
        ps_sT = ec(tc.tile_pool(name="ps_sT", bufs=1, space="PSUM"))
        ps_v = ec(tc.tile_pool(name="ps_v", bufs=2, space="PSUM"))
        ps_y = ec(tc.tile_pool(name="ps_y", bufs=1, space="PSUM"))

        ident = consts.tile([128, 128], F32R)
        nc.sync.dma_start(ident, ident_d)
        onesc = consts.tile([128, 2, 2], BF16)
        nc.sync.dma_start(onesc, onesc_d.rearrange("p (t k) -> p t k", t=2))
        negshift = consts.tile([128, 1], F32)
        nc.vector.memset(negshift, -SHIFT)
        eps_t = consts.tile([128, 1], F32)
        nc.vector.memset(eps_t, EPS)

        wqk = consts.tile([128, 2, 256], F32R)
        nc.sync.dma_start(wqk, wqk_d.rearrange("t p f -> p t f"))
        wv = consts.tile([128, 2, 256], F32R)
        nc.sync.dma_start(wv, wv_d.rearrange("t p f -> p t f"))

        if bqk_d is not None:
            bqk_sb = consts.tile([128, 2], F32)
            nc.sync.dma_start(bqk_sb, bqk_d.rearrange("t p -> p t"))
            import concourse.bass as bass
            bvf = consts.tile([128, 2, 256], F32)
            bv_b = bass.AP(tensor=bv_d.tensor, offset=bv_d.offset,
                           ap=[[0, 128], [0, 2], [1, 256]])
            nc.sync.dma_start(bvf, bv_b)

        def emit_tail(pv):
            """Slab tail, software-pipelined: y-matmuls, 1/Z normalize,
            skip-add, store."""
            E, vt, x_sb, s = pv["E"], pv["vt"], pv["x_sb"], pv["s"]
            p_y = ps_y.tile([128, 2, 512], F32)
            for it in (0, 1):
                for jt in (0, 1):
                    nc.tensor.matmul(
                        p_y[:, it, 0:258],
                        E[:, jt, it * 128:(it + 1) * 128],
                        vt[:, jt, 0:258],
                        start=(jt == 0), stop=(jt == 1))
            rZ = stat.tile([128, 2, 1], F32)
            nc.vector.reciprocal(rZ, p_y[:, :, 256:257])
            # out = x + y * rZ
            tmp = opool.tile([128, 2, 256], F32)
            for it in (0, 1):
                nc.vector.tensor_scalar_mul(tmp[:, it, :], p_y[:, it, 0:256],
                                            rZ[:, it, :])
            o_sb = opool.tile([128, 2, 256], F32)
            nc.gpsimd.tensor_tensor(out=o_sb, in0=tmp, in1=x_sb, op=ALU.add)
            nc.sync.dma_start(out_r[s], o_sb)

        def emit_scores(pq):
            """Scores stage, software-pipelined: s^T matmuls, exp."""
            qT = pq["qT"]
            p_sT = ps_sT.tile([128, 2, 256], F32)
            for jt in (0, 1):
                nc.tensor.matmul(
                    p_sT[:, jt, :],
                    qT[:, 1, jt * 128:(jt + 1) * 128],
                    qT[:, 0, :],
                    start=True, stop=True)
            E = epool.tile([128, 2, 256], BF16)
            nc.scalar.activation(out=E, in_=p_sT, func=AF.Exp,
                                 bias=negshift, scale=1.0)
            return {"E": E, "vt": pq["vt"], "x_sb": pq["x_sb"], "s": pq["s"]}

        def emit_front(s):
            """LN front-end for slab s: DMA, stats, rsqrt, normalize.
            Emitted one iteration ahead so the V/S LN chain never gates
            the slab's first PE op (the transposes)."""
            x_sb = xpool.tile([128, 2, 256], F32R)
            nc.sync.dma_start(x_sb, x_r[s])

            # LayerNorm stats per row (partition = w position)
            st = stat.tile([128, 2, 6], F32)
            mv = stat.tile([128, 2, 2], F32)
            for t in (0, 1):
                nc.vector.bn_stats(st[:, t, :], x_sb[:, t, :])
                nc.vector.bn_aggr(mv[:, t, :], st[:, t, :])
            # rs = rsqrt(var + eps) = exp(-0.5 * ln(var + eps))
            lnv = stat.tile([128, 2, 1], F32)
            nc.scalar.activation(out=lnv, in_=mv[:, :, 1:2], func=AF.Ln,
                                 bias=eps_t, scale=1.0)
            rs = stat.tile([128, 2, 1], F32)
            nc.scalar.activation(out=rs, in_=lnv, func=AF.Exp, scale=-0.5)

            # xn = (x - mean) * rs   (gamma/beta folded into weights on host)
            xn = xnpool.tile([128, 2, 256], F32R)
            for t in (0, 1):
                nc.vector.tensor_scalar(
                    out=xn[:, t, :], in0=x_sb[:, t, :],
                    scalar1=mv[:, t, 0:1], scalar2=rs[:, t, :],
                    op0=ALU.subtract, op1=ALU.mult)
            return {"x_sb": x_sb, "xn": xn}

        def emit_transp(fr):
            """PE transpose + PSUM->SBUF drain for a slab whose LN front-end
            was emitted earlier. Emitted one iteration ahead of consumption so
            qk/v never wait on same-iteration scalar-engine work."""
            xn = fr["xn"]
            p_xnT = ps_xnT.tile([128, 2, 256], F32R)
            for cc in (0, 1):
                for t in (0, 1):
                    nc.tensor.transpose(
                        p_xnT[:, cc, t * 128:(t + 1) * 128],
                        xn[:, t, cc * 128:(cc + 1) * 128], ident)
            xT = xtpool.tile([128, 2, 256], F32R)
            nc.scalar.copy(xT, p_xnT)
            return {"x_sb": fr["x_sb"], "xT": xT}

        # persistent vt ring: ones columns are pre-filled once; per-slab
        # copies only touch cols 0:256, so the Z ones never need rewriting
        vts = []
        for r in range(3):
            vt_r = vpool.tile([128, 2, 258], BF16)
            nc.vector.tensor_copy(vt_r[:, :, 256:258], onesc)
            vts.append(vt_r)

        # Steady-state PE order per iteration s:
        #   Y(s-2), SC(s-1), QK(s), V(s), T(s+1)
        # Every PE group consumes only elementwise products emitted in a
        # PREVIOUS iteration (except T <- xn, which has ~2us of V slack), so
        # the PE never waits on same-iteration Vector/Scalar work and stays
        # at max p-state.
        prev = None
        prevq = None
        cur = emit_transp(emit_front(0))
        front_next = emit_front(1)
        for s in range(NS):
            x_sb, xT = cur["x_sb"], cur["xT"]

            # front for s+2 FIRST: V leads with stats/xn and S with lnv/rs,
            # so neither sits behind PE-gated copies in its queue
            new_front = emit_front(s + 2) if s + 2 < NS else None

            if prev is not None:
                emit_tail(prev)
            if prevq is not None:
                prev = emit_scores(prevq)

            # T(s+1) in the MIDDLE of the PE order: its xT drain then has
            # QK(s)+V(s) plus the next iteration's Y+SC of PE cover before
            # QK(s+1) consumes it
            nxt_cur = emit_transp(front_next) if s + 1 < NS else None
            front_next = new_front

            # qk^T = Wqk^T @ xn^T
            p_qk = ps_qk.tile([128, 2, 256], F32)
            for blk in (0, 1):
                for cc in (0, 1):
                    nc.tensor.matmul(
                        p_qk[:, blk, :],
                        wqk[:, cc, blk * 128:(blk + 1) * 128],
                        xT[:, cc, :],
                        start=(cc == 0), stop=(cc == 1))
            if bqk_d is not None:
                for blk in (0, 1):
                    nc.vector.tensor_scalar(
                        out=p_qk[:, blk, :], in0=p_qk[:, blk, :],
                        scalar1=bqk_sb[:, blk:blk + 1], scalar2=None,
                        op0=ALU.add)
            qT = qkpool.tile([128, 2, 256], F32R)
            nc.scalar.copy(qT, p_qk)

            # v = xn @ Wv, with a ones column appended for Z accumulation
            p_v = ps_v.tile([128, 2, 256], F32)
            for jt in (0, 1):
                for cc in (0, 1):
                    nc.tensor.matmul(
                        p_v[:, jt, :],
                        xT[:, cc, jt * 128:(jt + 1) * 128],
                        wv[:, cc, :],
                        start=(cc == 0), stop=(cc == 1))
            vt = vts[s % 3]
            if bv_d is not None:
                nc.vector.tensor_tensor(out=vt[:, :, 0:256], in0=p_v, in1=bvf,
                                        op=ALU.add)
            else:
                nc.scalar.copy(vt[:, :, 0:256], p_v)

            cur = nxt_cur

            prevq = {"qT": qT, "vt": vt, "x_sb": x_sb, "s": s}
        emit_tail(prev)
        prev = emit_scores(prevq)
        emit_tail(prev)


def _install_ntff_hook():
    """Register the axon NTFF profiling hook. Trace-only; best-effort."""
    try:
        import types

        import antenv

        if getattr(antenv, "axon_hooks", None) is not None:
            return
        mod = types.ModuleType("antenv.axon_hooks")
        _h = [None]
        mod.set_axon_ntff_profile_hook = lambda h: _h.__setitem__(0, h)
        mod.get_axon_ntff_profile_hook = lambda: _h[0]
        sys.modules["antenv.axon_hooks"] = mod
        antenv.axon_hooks = mod
        from trn_agent_boot.trn_boot import _ntff_profile_via_ctypes

        hook = _ntff_profile_via_ctypes("/opt/axon/libaxon_pjrt.so")
        if hook is not None:
            mod.set_axon_ntff_profile_hook(hook)
    except Exception as e:  # noqa: BLE001
        print(f"ntff hook install failed (timing unavailable): {e}")


def kernel(x, ln_gamma, ln_beta, W_qkv):
    x = np.asarray(x, dtype=np.float32)
    ln_gamma = np.asarray(ln_gamma, dtype=np.float32)
    ln_beta = np.asarray(ln_beta, dtype=np.float32)
    W_qkv = np.asarray(W_qkv, dtype=np.float32)
    assert x.shape == (B, H, W, C) and W_qkv.shape == (C, F2)

    # fold gamma/beta into the projection (1x1 conv has no bias of its own)
    Wp = (ln_gamma.astype(np.float64)[:, None] * W_qkv.astype(np.float64))
    bW = (ln_beta.astype(np.float64) @ W_qkv.astype(np.float64)).astype(np.float32)
    with_bias = bool(np.any(bW != 0.0))

    key = with_bias
    if key not in _NC_CACHE:
        _NC_CACHE[key] = _build(with_bias)
    nc = _NC_CACHE[key]

    wqk = np.ascontiguousarray(
        Wp[:, :256].astype(np.float32).reshape(2, 128, 256))
    wv = np.ascontiguousarray(
        Wp[:, 256:].astype(np.float32).reshape(2, 128, 256))
    ident = np.eye(128, dtype=np.float32)
    import ml_dtypes
    onesc = np.ones((128, 4), dtype=ml_dtypes.bfloat16)
    in_maps = []
    for b in range(B):
        m = {
            "x": np.ascontiguousarray(x[b]),
            "wqk": wqk,
            "wv": wv,
            "ident_in": ident,
            "onesc_in": onesc,
        }
        if with_bias:
            m["bqk"] = np.ascontiguousarray(bW[:256].reshape(2, 128))
            m["bv"] = np.ascontiguousarray(bW[256:])
        in_maps.append(m)

    trace = os.environ.get("KERNEL_TRACE", "") == "1"
    if trace:
        _install_ntff_hook()
    res = run_bass_kernel_spmd(nc, in_maps, core_ids=list(range(B)), trace=trace)
    if trace and res.exec_time_ns is not None:
        print(f"HW exec time: {res.exec_time_ns} ns")
        if res.instructions_and_trace is not None:
            print(f"trace: {res.instructions_and_trace[1]}")
    out = np.stack([res.results[b]["out"] for b in range(B)], axis=0)
    return out.reshape(B, H, W, C).astype(np.float32, copy=False)
